# revision 1
# baseline (speedup 1.0000x reference)
"""Trainium2 Bass kernel for topk_masking (nn_CustomModule_8065948582484).

Reference semantics (per batch b):
  idx[b,f] = argmax(score[b,f,:196])                 (first index on ties)
  mask grows from a fixed prior region on a 14x14 grid; at frame f the
  argmax cell is added iff it is 4-adjacent to the current mask.
  out = [ones(B,1), masks frame-major] -> [B, 1+16*196] fp32.

Strategy (pure batch data-parallel across 8 cores, 2048 batches/core,
16 tiles of 128 batches on SBUF partitions, processed in groups of 8):
  1. argmax via prefix-max scan with per-frame guard resets (DVE), then
     idx = popcount(prefix_max < max) on ScalarE's sign+accumulator --
     exact incl. first-index tie semantics.
  2. row r via popcount over row-end prefix-max slots; c = idx - 14r.
  3. the per-frame "added" recurrence runs on a tiny 16-node adjacency
     graph; graph build + recurrence are batched across the 8-tile group.
  4. masks: one-hot (tensor_scalar 4x) + running max (tensor_tensor 2x).
  5. ScalarE converts bf16 masks -> fp32 output; DMA in/out overlapped.
"""

import os
import sys

import numpy as np

for _p in ("/opt/trn_rl_repo",):
    if _p not in sys.path:
        sys.path.insert(0, _p)

from concourse import bacc, mybir, tile  # noqa: E402
from concourse.bass_utils import run_bass_kernel_spmd  # noqa: E402

B, F, P = 16384, 16, 196
N = 14  # grid side
S = P + 1  # slots per frame in the scan layout (guard + 196)
NCORES = 8
BLOC = B // NCORES  # 2048
NT = BLOC // 128  # 16 tiles per core
GS = 8  # tiles per batched phase-2 group
NG = NT // GS

ALU = mybir.AluOpType
AX = mybir.AxisListType
F32 = mybir.dt.float32
BF16 = mybir.dt.bfloat16
ACT = mybir.ActivationFunctionType
BIG = 1e30


def build_nc():
    nc = bacc.Bacc(trn_type="TRN2", target_bir_lowering=False)
    score_d = nc.declare_dram_parameter("score", [BLOC, F, P], F32, isOutput=False)
    out_d = nc.declare_dram_parameter("out", [BLOC, 1 + F * P], F32, isOutput=True)

    with tile.TileContext(nc) as tc:
        with (
            tc.tile_pool(name="consts", bufs=1) as cpool,
            tc.tile_pool(name="big", bufs=2) as bpool,
            tc.tile_pool(name="grp", bufs=2) as gpool,
        ):
            # ---- constants ----
            iotap = cpool.tile([128, P], BF16, name="iotap")
            nc.gpsimd.iota(
                iotap[:],
                pattern=[[1, P]],
                base=0,
                channel_multiplier=0,
                allow_small_or_imprecise_dtypes=True,
            )
            prior = cpool.tile([128, P], BF16, name="prior")
            nc.vector.memset(prior[:], 0.0)
            priorv = prior.rearrange("q (r c) -> q r c", r=N)
            nc.vector.memset(priorv[:, 4:14, 2:12], 1.0)
            d1 = cpool.tile([128, F * S], BF16, name="d1")
            nc.vector.memset(d1[:], BIG)
            d1v = d1.rearrange("q (f s) -> q f s", f=F)
            nc.vector.memset(d1v[:, :, 0:1], -BIG)

            for g in range(NG):
                # group tiles: [128, F, GS] layouts (frame-major, tile inner)
                idxa = gpool.tile([128, F, GS], F32, tag="idxa", name="idxa")
                rr = gpool.tile([128, F, GS], F32, tag="rr", name="rr")
                runs = []

                # ---- phase A: per-tile load / scan / counts ----
                for j in range(GS):
                    t = g * GS + j
                    r0 = t * 128
                    sc = bpool.tile([128, F * S], F32, tag="sc", name="sc", bufs=3)
                    scv = sc.rearrange("q (f s) -> q f s", f=F)
                    nc.vector.memset(scv[:, :, 0:1], -BIG)
                    nc.sync.dma_start(
                        out=scv[:, :, 1:S], in_=score_d[r0 : r0 + 128]
                    )
                    run = bpool.tile([128, F * S], F32, tag="run", name="run", bufs=3)
                    nc.vector.tensor_tensor_scan(
                        run[:], sc[:], d1[:], 0.0, ALU.max, ALU.min
                    )
                    runv = run.rearrange("q (f s) -> q f s", f=F)
                    runs.append(runv)
                    # idx = #positions with prefix-max below the frame max:
                    # ScalarE sign(m - prefix) in {1,0}, summed by its accumulator
                    for f in range(F):
                        nc.scalar.activation(
                            sjunk := gpool.tile(
                                [128, P], BF16, tag="sjunk", name="sjunk", bufs=2
                            ),
                            runv[:, f, 1:S],
                            ACT.Sign,
                            bias=runv[:, f, P : P + 1],
                            scale=-1.0,
                            accum_out=idxa[:, f, j : j + 1],
                        )
                    # r = #row-end slots with prefix-max below the frame max
                    rowends = runv[:, :, N : S : N]
                    m_b = runv[:, :, P : P + 1].broadcast_to([128, F, N])
                    rlt = gpool.tile([128, F, N], BF16, tag="rlt", name="rlt", bufs=2)
                    nc.vector.tensor_tensor(rlt[:], rowends, m_b, ALU.is_lt)
                    nc.vector.tensor_reduce(
                        rr[:, :, j], rlt[:], axis=AX.X, op=ALU.add
                    )

                # ---- phase B: batched small compute for the whole group ----
                # c = idx - 14 r ; v = 16 r + c  (pitch-16 id: adj <=> |dv| in {1,16})
                cc = gpool.tile([128, F, GS], F32, tag="cc", name="cc")
                nc.vector.scalar_tensor_tensor(
                    cc[:], rr[:], -float(N), idxa[:], ALU.mult, ALU.add
                )
                vv = gpool.tile([128, F, GS], F32, tag="vv", name="vv")
                nc.vector.scalar_tensor_tensor(
                    vv[:], rr[:], 16.0, cc[:], ALU.mult, ALU.add
                )
                vb = gpool.tile([128, F, GS], BF16, tag="vb", name="vb")
                nc.vector.tensor_copy(vb[:], vv[:])

                # pairwise |v_e - v_f| -> adjacency gg[e,f,t]
                dv = gpool.tile([128, F, F, GS], BF16, tag="dv", name="dv")
                nc.vector.tensor_tensor(
                    dv[:],
                    vb.unsqueeze(2).broadcast_to([128, F, F, GS]),
                    vb.unsqueeze(1).broadcast_to([128, F, F, GS]),
                    ALU.subtract,
                )
                ndv = gpool.tile([128, F, F, GS], BF16, tag="ndv", name="ndv")
                nc.vector.tensor_scalar(ndv[:], dv[:], -1.0, None, ALU.mult)
                adv = gpool.tile([128, F, F, GS], BF16, tag="adv", name="adv")
                nc.vector.tensor_tensor(adv[:], dv[:], ndv[:], ALU.max)
                g1 = gpool.tile([128, F, F, GS], BF16, tag="g1", name="g1")
                nc.vector.tensor_scalar(g1[:], adv[:], 1.0, None, ALU.is_equal)
                g16 = gpool.tile([128, F, F, GS], BF16, tag="g16", name="g16")
                nc.vector.tensor_scalar(g16[:], adv[:], 16.0, None, ALU.is_equal)
                gg = gpool.tile([128, F, F, GS], BF16, tag="gg", name="gg")
                nc.vector.tensor_tensor(gg[:], g1[:], g16[:], ALU.add)

                # A = (r>=3 & 2<=c<=11) | (r>=4 & 1<=c<=12)
                u3 = gpool.tile([128, F, GS], BF16, tag="u3", name="u3")
                nc.vector.tensor_scalar(u3[:], rr[:], 3.0, None, ALU.is_ge)
                u4 = gpool.tile([128, F, GS], BF16, tag="u4", name="u4")
                nc.vector.tensor_scalar(u4[:], rr[:], 4.0, None, ALU.is_ge)
                cm2 = gpool.tile([128, F, GS], F32, tag="cm2", name="cm2")
                nc.vector.tensor_scalar(cm2[:], cc[:], 2.0, None, ALU.subtract)
                q1 = gpool.tile([128, F, GS], F32, tag="q1", name="q1")
                nc.vector.scalar_tensor_tensor(
                    q1[:], cc[:], -11.0, cm2[:], ALU.add, ALU.mult
                )
                b1 = gpool.tile([128, F, GS], BF16, tag="b1", name="b1")
                nc.vector.tensor_scalar(b1[:], q1[:], 0.0, None, ALU.is_le)
                cm1 = gpool.tile([128, F, GS], F32, tag="cm1", name="cm1")
                nc.vector.tensor_scalar(cm1[:], cc[:], 1.0, None, ALU.subtract)
                q2 = gpool.tile([128, F, GS], F32, tag="q2", name="q2")
                nc.vector.scalar_tensor_tensor(
                    q2[:], cc[:], -12.0, cm1[:], ALU.add, ALU.mult
                )
                b2 = gpool.tile([128, F, GS], BF16, tag="b2", name="b2")
                nc.vector.tensor_scalar(b2[:], q2[:], 0.0, None, ALU.is_le)
                t1 = gpool.tile([128, F, GS], BF16, tag="t1", name="t1")
                nc.vector.tensor_tensor(t1[:], u3[:], b1[:], ALU.logical_and)
                t2 = gpool.tile([128, F, GS], BF16, tag="t2", name="t2")
                nc.vector.tensor_tensor(t2[:], u4[:], b2[:], ALU.logical_and)
                aa = gpool.tile([128, F, GS], F32, tag="aa", name="aa")
                nc.vector.tensor_tensor(aa[:], t1[:], t2[:], ALU.logical_or)

                # sequential added-recurrence, batched over the group:
                # added[f] = max(A[f], max_e added[e]*G[e,f])
                added = gpool.tile([128, F, GS], BF16, tag="added", name="added")
                nc.vector.memset(added[:], 0.0)
                t16 = gpool.tile([128, F, GS], BF16, tag="t16", name="t16")
                mx = gpool.tile([128, GS], F32, tag="mx", name="mx")
                for f in range(F):
                    nc.vector.tensor_tensor(
                        t16[:], added[:], gg[:, :, f, :], ALU.mult
                    )
                    t16v = t16.rearrange("q e t -> q t e")
                    nc.vector.tensor_reduce(mx[:], t16v, axis=AX.X, op=ALU.max)
                    nc.vector.tensor_tensor(
                        added[:, f, :], mx[:], aa[:, f, :], ALU.max
                    )

                # idxm[f] = added[f] ? idx[f] : -1   (-1 never matches iotap)
                ip1 = gpool.tile([128, F, GS], F32, tag="ip1", name="ip1")
                nc.vector.tensor_scalar(ip1[:], idxa[:], 1.0, None, ALU.add)
                idxm = gpool.tile([128, F, GS], F32, tag="idxm", name="idxm")
                nc.vector.tensor_tensor(idxm[:], ip1[:], added[:], ALU.mult)
                nc.vector.tensor_scalar(idxm[:], idxm[:], 1.0, None, ALU.subtract)

                # ---- phase C: per-tile mask build / convert / store ----
                for j in range(GS):
                    t = g * GS + j
                    r0 = t * 128
                    masks = bpool.tile(
                        [128, F * P], BF16, tag="masks", name="masks"
                    )
                    for f in range(F):
                        oh = gpool.tile([128, P], BF16, tag="oh", name="oh", bufs=3)
                        nc.vector.tensor_scalar(
                            oh[:], iotap[:], idxm[:, f, j : j + 1], None, ALU.is_equal
                        )
                        prev = prior[:] if f == 0 else masks[:, (f - 1) * P : f * P]
                        nc.vector.tensor_tensor(
                            masks[:, f * P : (f + 1) * P], prev, oh[:], ALU.max
                        )
                    out_t = bpool.tile([128, 1 + F * P], F32, tag="out", name="out_t")
                    nc.vector.memset(out_t[:, 0:1], 1.0)
                    nc.scalar.activation(out_t[:, 1 : 1 + F * P], masks[:], ACT.Copy)
                    nc.sync.dma_start(out=out_d[r0 : r0 + 128, :], in_=out_t[:])

    nc.compile()
    return nc


_nc = None


def _get_nc():
    global _nc
    if _nc is None:
        _nc = build_nc()
    return _nc


def kernel(score, topn=196):
    score = np.ascontiguousarray(np.asarray(score, dtype=np.float32)).reshape(B, F, P)
    nc = _get_nc()
    in_maps = [
        {"score": score[i * BLOC : (i + 1) * BLOC]} for i in range(NCORES)
    ]
    res = run_bass_kernel_spmd(nc, in_maps, list(range(NCORES)))
    out = np.concatenate([res.results[i]["out"] for i in range(NCORES)], axis=0)
    return out

